# revision 1
# baseline (speedup 1.0000x reference)
"""Chamfer loss kernel for Trainium2 (8 NeuronCores, batch-data-parallel).

Math: for each batch b, dist_sq[n,m] = |p3[n]|^2 + |q3[m]|^2 - 2 p3[n].q3[m].
The reference takes sqrt(max(dist_sq,0)+eps), dual-axis mins, then sums.
sqrt/max/+eps are monotone, so min commutes with them: the device computes
min_m dist_sq (per n) and min_n dist_sq (per m); the host finishes.

ONE-PASS design (vs the previous two-pass): each 128x1024 distance tile is
computed ONCE in PSUM (as -dist_sq/2 so every reduction is a max), read ONCE
by the Scalar engine (the only PSUM read, 2 ACTIVATE per round), and both
reduction directions are extracted from the bf16 SBUF copy:
  - row-min (over m): TENSOR_TENSOR is the only DVE op with a working 2x
    perf mode on this silicon (tensor_reduce / tensor_scalar / activation-
    accum all measured 1x), so fold 1024->128 with three 2x TT-maxes per
    round, then one small 1x reduce into a bf16 [128,4] result slice.
  - col-min (over n): running 2x TT-max into a [128, 4x1024] per-quad
    accumulator (first two S tiles seed the chain, 7 TTs per quad),
    finished by gpsimd partition_all_reduce(max) across the 128 partitions
    (~13.6us per quad, overlapped); one [1,1024] row DMA'd out per batch.
Engine busy per core (measured): DVE ~190us (bottleneck), ACT ~127us,
PE array ~39us wall, Pool ~58us of partition reduces.
Keep the gpsimd partition-reduce volume low: Pool shares its SBUF port
with DVE, so doubling p_a_r traffic measurably slows every DVE op.

Matmul: one K=24 bf16 matmul per (batch, 128-row tile, 512-col bank) writes
PSUM = -dist_sq/2 directly. K rows: a 3-level bf16 split (h+l+r) of the
3-vectors with pairings hh+hl+lh+hr+rh+ll (~1e-6 abs error; fp32 matmul is
4x slower), plus 3-term splits of the (negated, halved) point norms against
`ones` rows. 4 batches pack into the 4 PE row-groups (operands at
partitions 32g..32g+KROWS) so 4 matmuls run concurrently.

Output: res_row (128, 128) fp32 = per-partition row maxes of -dist_sq/2;
res_col (16, 1024) fp32 = per-batch col maxes. Host decodes, applies
sqrt(max(-2v,0)+eps), and sums across cores in float64.
"""

import numpy as np

import concourse.bass as bass  # noqa: F401  (bass types used via bacc/tile)
import concourse.mybir as mybir
import concourse.tile as tile
from concourse import bacc
from concourse.bass_isa import ReduceOp
from concourse.bass_utils import run_bass_kernel_spmd

B, N, M = 128, 1024, 1024
NCORES = 8
BPC = B // NCORES  # 16 batches per core
NQUAD = BPC // 4  # 4 quads of 4 batches
F32 = mybir.dt.float32
BF16 = mybir.dt.bfloat16
KROWS = 24  # bf16 3-level split: 18 cross rows + 3 qn rows + 3 pn rows

_CACHE = {}
NEG_BIG = -3.0e38



def _body(tc, dram, res_row_d, res_col_d):
    nc = tc.nc
    with (
        tc.tile_pool(name="stacks", bufs=1) as stacks,
        tc.tile_pool(name="scratchp", bufs=1) as scratchp,
        tc.tile_pool(name="resp", bufs=1) as resp,
        tc.tile_pool(name="psump", bufs=1, space="PSUM") as psump,
    ):
        stk = {}
        # quad-0 slices of the lhsT/rhs stacks first so the first rounds'
        # matmuls are not gated on the whole prologue transfer
        for nm in ("ap_s", "bq_s"):
            t = stacks.tile([128, NQUAD, 1024], BF16, name=nm + "_t", tag=nm + "_t")
            stk[nm] = t
        # finest-first: rhs quad 0, then lhsT columns for round 0, then the
        # rest of quad 0, then quads 1-3.  The round-0 critical transfers
        # spread across three DGE engines (three DMA queues) so the first
        # matmul is not gated on one 22.5 GB/s queue.
        dges = [nc.sync, nc.scalar]
        for j in range(2):
            for g in range(4):
                dges[g % 2].dma_start(
                    out=stk["bq_s"][
                        32 * g : 32 * g + KROWS, 0:1, 512 * j : 512 * (j + 1)
                    ],
                    in_=dram["bq_s"][g, :, 0:1, 512 * j : 512 * (j + 1)],
                )
        for g in range(4):
            dges[g % 2].dma_start(
                out=stk["ap_s"][32 * g : 32 * g + KROWS, 0:1, 0:128],
                in_=dram["ap_s"][g, :, 0:1, 0:128],
            )
        for g in range(4):
            nc.sync.dma_start(
                out=stk["ap_s"][32 * g : 32 * g + KROWS, 0:1, 128:1024],
                in_=dram["ap_s"][g, :, 0:1, 128:1024],
            )
        for nm in ("ap_s", "bq_s"):
            t = stk[nm]
            for g in range(4):
                nc.sync.dma_start(
                    out=t[32 * g : 32 * g + KROWS, 1:NQUAD],
                    in_=dram[nm][g, :, 1:NQUAD],
                )

        # [128, i(8), b_loc(16)]; flat col = 16*i + b_loc, as the host expects
        res_row = resp.tile([128, 8, BPC], BF16, name="res_row", tag="res_row")

        A, Bs = stk["ap_s"], stk["bq_s"]

        def _colmax_finish(t_i, half, chain_root):
            # cross-partition col maxes of one half-chain root; one row out
            # per batch into res_col row (half*BPC + b_loc); host maxes the
            # two halves
            parout = scratchp.tile(
                [128, 4, 1024], F32, name="par", tag="par", bufs=2
            )
            nc.gpsimd.partition_all_reduce(
                parout.rearrange("p a b -> p (a b)"),
                chain_root.rearrange("p a b c -> p (a b c)"),
                128,
                ReduceOp.max,
            )
            for g in range(4):
                b_loc = 4 * t_i + g
                nc.sync.dma_start(
                    out=res_col_d[half * BPC + b_loc : half * BPC + b_loc + 1, :],
                    in_=parout[0:1, g, :],
                )

        for t_i in range(NQUAD):
            sfirst = None  # first S tile of the current half-chain
            cprev = None  # col accumulator (current half-chain)
            for i in range(8):
                pr = [
                    psump.tile([128, 2, 2, 512], F32, name=f"pr{h}", tag=f"pr{h}")
                    for h in range(2)
                ]
                for j in range(2):
                    for g in range(4):
                        nc.tensor.matmul(
                            pr[g // 2][:, g % 2, j, :],
                            A[32 * g : 32 * g + KROWS, t_i, 128 * i : 128 * (i + 1)],
                            Bs[32 * g : 32 * g + KROWS, t_i, 512 * j : 512 * (j + 1)],
                            start=True,
                            stop=True,
                            tile_position=(32 * g, 0),
                        )
                # evacuate the round (4 batches x 1024 cols) once, per pair:
                # the only PSUM read; everything downstream is bf16 SBUF
                s = scratchp.tile(
                    [128, 4, 2, 512], BF16, name="s", tag="s", bufs=3
                )
                for h in range(2):
                    nc.scalar.copy(s[:, 2 * h : 2 * h + 2], pr[h][:, :, :, :])
                # row maxes for all 4 batches: TENSOR_TENSOR is the only op
                # with a 2x perf mode on this silicon (reduce/tensor_scalar
                # measured 1x).  First fold (1024->512) runs per round into
                # a round-pair tile; the lower fold levels and the final 1x
                # reduce batch TWO rounds per instruction to halve small-op
                # overhead.
                if i % 2 == 0:
                    upair = scratchp.tile(
                        [128, 2, 4, 512], BF16, name="U", tag="U", bufs=2
                    )
                nc.vector.tensor_tensor(
                    out=upair[:, i % 2], in0=s[:, :, 0, :], in1=s[:, :, 1, :],
                    op=mybir.AluOpType.max,
                )
                if i % 2 == 1:
                    w = scratchp.tile(
                        [128, 2, 4, 256], BF16, name="w", tag="w", bufs=2
                    )
                    nc.vector.tensor_tensor(
                        out=w, in0=upair[:, :, :, 0:256], in1=upair[:, :, :, 256:512],
                        op=mybir.AluOpType.max,
                    )
                    x = scratchp.tile(
                        [128, 2, 4, 128], BF16, name="x", tag="x", bufs=2
                    )
                    nc.vector.tensor_tensor(
                        out=x, in0=w[:, :, :, 0:128], in1=w[:, :, :, 128:256],
                        op=mybir.AluOpType.max,
                    )
                    nc.vector.tensor_reduce(
                        out=res_row[:, i - 1 : i + 1, 4 * t_i : 4 * t_i + 4],
                        in_=x,
                        axis=mybir.AxisListType.X,
                        op=mybir.AluOpType.max,
                    )
                # col accumulate (4 batches wide): C = max(C, round tiles);
                # one chain per quad (splitting doubles the gpsimd
                # partition-reduce volume, which steals the SBUF port DVE
                # shares with it and slows every DVE op — measured net loss)
                if i == 0:
                    sfirst = s
                else:
                    cnew = scratchp.tile(
                        [128, 4, 2, 512], BF16, name="C", tag="C", bufs=2
                    )
                    nc.vector.tensor_tensor(
                        out=cnew,
                        in0=s,
                        in1=sfirst if i == 1 else cprev,
                        op=mybir.AluOpType.max,
                    )
                    cprev = cnew
                    if i == 7:
                        _colmax_finish(t_i, 0, cprev)

        nc.sync.dma_start(out=res_row_d, in_=res_row)


def _build_nc():
    if "nc" in _CACHE:
        return _CACHE["nc"]
    nc = bacc.Bacc(
        "TRN2", target_bir_lowering=False, debug=False, num_devices=NCORES
    )
    dram = {}
    for nm in ("ap_s", "bq_s"):
        dram[nm] = nc.dram_tensor(
            nm, (4, KROWS, NQUAD, 1024), BF16, kind="ExternalInput"
        ).ap()
    res_row_d = nc.dram_tensor(
        "res_row", (128, BPC * 8), BF16, kind="ExternalOutput"
    ).ap()
    res_col_d = nc.dram_tensor(
        "res_col", (BPC, 1024), F32, kind="ExternalOutput"
    ).ap()
    with tile.TileContext(nc) as tc:
        _body(tc, dram, res_row_d, res_col_d)
    nc.compile()
    _CACHE["nc"] = nc
    return nc


def _split3(x):
    """Split fp32 into 3 bf16 terms (x ~= h + l + r, error ~2^-27 |x|)."""
    import ml_dtypes

    bf = ml_dtypes.bfloat16
    h = x.astype(bf)
    l = (x - h.astype(np.float32)).astype(bf)
    r = (x - h.astype(np.float32) - l.astype(np.float32)).astype(bf)
    return h, l, r


def _host_stacks(x3, xn, lhs):
    """x3: (BPC, 1024, 3), xn: (BPC, 1024) -> (4, KROWS, NQUAD, 1024) bf16.

    Layout [g, k, t, n]: batch 4*t + g lives in PE row-group g (SBUF
    partitions 32g+k). With h/l/r the bf16 3-level split, the K pairing
    slots are
      cross (x3): lhsT [h h l h r l], rhs [h l h r h l]  (x3 comps each)
      norms: lhsT [1 1 1 h(-xn/2) l r], rhs [h(-yn/2) l r 1 1 1]
    so lhsT[k]*rhs[k] accumulates the cross terms MINUS the norm halves ->
    PSUM = -dist_sq/2 with ~1e-6 absolute error."""
    import ml_dtypes

    bf = ml_dtypes.bfloat16
    out = np.empty((NQUAD, 4, KROWS, 1024), bf)  # [t, g, k, n]
    x3t = np.transpose(x3.reshape(NQUAD, 4, 1024, 3), (0, 1, 3, 2))  # (t,g,3,n)
    h3, l3, r3 = _split3(x3t)
    hn, ln, rn = _split3((xn * -0.5).reshape(NQUAD, 4, 1024))
    one = np.asarray(1.0, bf)
    if lhs:
        cross = (h3, h3, l3, h3, r3, l3)
        norm = (one, one, one, hn, ln, rn)
    else:
        cross = (h3, l3, h3, r3, h3, l3)
        norm = (hn, ln, rn, one, one, one)
    for s in range(6):
        out[:, :, 3 * s : 3 * s + 3] = cross[s]
        out[:, :, 18 + s] = norm[s]
    return np.ascontiguousarray(np.transpose(out, (1, 2, 0, 3)))


def _run(p, q, trace=False, tmpdir=None):
    p = np.asarray(p)
    q = np.asarray(q)
    assert p.shape == (B, N, 4) and q.shape == (B, M, 4)
    p3 = np.ascontiguousarray(p[:, :, 1:], dtype=np.float32)
    q3 = np.ascontiguousarray(q[:, :, 1:], dtype=np.float32)
    pn = np.einsum("bnc,bnc->bn", p3, p3)
    qn = np.einsum("bmc,bmc->bm", q3, q3)

    in_maps = []
    for c in range(NCORES):
        sl = slice(BPC * c, BPC * (c + 1))
        in_maps.append(
            {
                "ap_s": _host_stacks(p3[sl], pn[sl], lhs=True),
                "bq_s": _host_stacks(q3[sl], qn[sl], lhs=False),
            }
        )

    nc = _build_nc()
    kw = {}
    if trace:
        kw = {"trace": True, "tmpdir": tmpdir}
    rb = run_bass_kernel_spmd(nc, in_maps, core_ids=list(range(NCORES)), **kw)

    total = 0.0
    for c in range(NCORES):
        # values are max(-dist_sq/2) -> dist_sq = -2v
        vrow = rb.results[c]["res_row"].astype(np.float64)  # (128, 128)
        vcol = rb.results[c]["res_col"].astype(np.float64)  # (16, 1024)
        for v in (vrow, vcol):
            d_sq = np.maximum(-2.0 * v, 0.0) + 1e-16
            total += np.sqrt(d_sq).sum()
    out = np.float32(total / 2.0)
    return out, rb


def kernel(p, q):
    out, _ = _run(p, q)
    return out



# revision 2
# speedup vs baseline: 1.2471x; 1.2471x over previous
"""Chamfer loss kernel for Trainium2 (8 NeuronCores, batch-data-parallel).

Math: for each batch b, dist_sq[n,m] = |p3[n]|^2 + |q3[m]|^2 - 2 p3[n].q3[m].
The reference takes sqrt(max(dist_sq,0)+eps), dual-axis mins, then sums.
sqrt/max/+eps are monotone, so min commutes with them: the device computes
min_m dist_sq (per n) and min_n dist_sq (per m); the host finishes.
PSUM holds v = -dist_sq/2 (= p.q - pn/2 - qn/2) so every reduction is a max.

Pipeline (per core: 16 batches = 4 quads of 4; per quad 8 row-tiles of 128):
 - Half-round = (row-tile rt, col-half j): 4 matmuls (one per PE row-group,
   one PSUM bank each) write pr = [128, 4 batches, 512 cols] fp32. Two pr
   tiles rotate over the 8 PSUM banks so round k+1's matmuls overlap round
   k's evacuation.
 - Scalar engine: one ACTIVATE per half-round copies pr -> s[:, :, j, :]
   (bf16). The only PSUM read; ~2.08us each, 64 total.
 - Row mins (DVE, all bf16 2x TENSOR_TENSOR): fold1 = max(s_j0, s_j1) per
   row-tile into a 4-row-tile batch tile; every 4 row-tiles w/x/y fold
   512->64 and one 1x TENSOR_REDUCE writes res_row[:, rt, batch].
 - Col mins: 6 TT per quad fold the 8 s tiles into 2 accumulators
   (rt 0-3 and rt 4-7), each DMA'd to DRAM. The final 128-partition max
   runs on the HOST via a uint16 trick: for bf16 values <= 0, float max =
   unsigned-int min, so numpy's uint16 min decodes it (any positive values
   are ~1e-6 matmul-rounding artifacts; uint16-min ranks them above all
   negatives, max error ~1e-6 on dist_sq). No gpsimd => no SBUF-port
   contention with DVE and no partition-reduce tail.
 - K=13 bf16 matmul rows: 2-level split (h+l) with pairings hh+hl+lh for
   the 3 cross components (9 rows) plus h/l splits of -pn/2, -qn/2 against
   `ones` rows (4 rows). Dropped l*l term ~2^-18 => ~2e-5 abs on dist_sq.

Engine busy per core (predicted): DVE ~146us, ACT ~143us, PE ~40us wall,
no gpsimd. Output: res_row (128, 8*16) bf16 row maxes; col_acc
(NQUAD*2, 128, 4096) bf16 partial col maxes. Host decodes both, applies
sqrt(max(-2v,0)+eps), sums in float64 across cores.
"""

import numpy as np

import concourse.bass as bass  # noqa: F401  (bass types used via bacc/tile)
import concourse.mybir as mybir
import concourse.tile as tile
from concourse import bacc
from concourse.bass_utils import run_bass_kernel_spmd

B, N, M = 128, 1024, 1024
NCORES = 8
BPC = B // NCORES  # 16 batches per core
NQUAD = BPC // 4  # 4 quads of 4 batches
F32 = mybir.dt.float32
BF16 = mybir.dt.bfloat16
KROWS = 13  # 2-level bf16 split: 9 cross rows + 2 qn rows + 2 pn rows
ACC = 2  # col accumulators DMA'd to host per quad

_CACHE = {}
MAX = mybir.AluOpType.max


def _body(tc, dram, res_row_d, col_d):
    nc = tc.nc
    with (
        tc.tile_pool(name="stacks", bufs=1) as stacks,
        tc.tile_pool(name="scratchp", bufs=1) as scratchp,
        tc.tile_pool(name="resp", bufs=1) as resp,
        tc.tile_pool(name="psump", bufs=1, space="PSUM") as psump,
    ):
        stk = {}
        for nm in ("ap_s", "bq_s"):
            stk[nm] = stacks.tile(
                [128, NQUAD, 1024], BF16, name=nm + "_t", tag=nm + "_t"
            )
        # finest-first: operands for half-round (rt0, j0) first, split
        # across the two HWDGE queues; then the rest of quad 0; then
        # quads 1-3 in bulk.
        dges = [nc.sync, nc.scalar]
        for g in range(4):
            dges[g % 2].dma_start(
                out=stk["bq_s"][32 * g : 32 * g + KROWS, 0:1, 0:512],
                in_=dram["bq_s"][g, :, 0:1, 0:512],
            )
            dges[(g + 1) % 2].dma_start(
                out=stk["ap_s"][32 * g : 32 * g + KROWS, 0:1, 0:128],
                in_=dram["ap_s"][g, :, 0:1, 0:128],
            )
        for g in range(4):
            dges[g % 2].dma_start(
                out=stk["bq_s"][32 * g : 32 * g + KROWS, 0:1, 512:1024],
                in_=dram["bq_s"][g, :, 0:1, 512:1024],
            )
        for g in range(4):
            dges[g % 2].dma_start(
                out=stk["ap_s"][32 * g : 32 * g + KROWS, 0:1, 128:1024],
                in_=dram["ap_s"][g, :, 0:1, 128:1024],
            )
        for nm in ("ap_s", "bq_s"):
            for g in range(4):
                dges[g % 2].dma_start(
                    out=stk[nm][32 * g : 32 * g + KROWS, 1:NQUAD],
                    in_=dram[nm][g, :, 1:NQUAD],
                )

        # [128, rt(8), b_loc(16)]; flat col = 16*rt + b_loc (host layout)
        res_row = resp.tile([128, 8, BPC], BF16, name="res_row", tag="res_row")

        A, Bs = stk["ap_s"], stk["bq_s"]

        for t_i in range(NQUAD):
            s_prev = None
            pairs = []
            u = None
            for rt in range(8):
                s = scratchp.tile([128, 4, 2, 512], BF16, name="s", tag="s", bufs=4)
                for j in range(2):
                    pr = psump.tile([128, 4, 512], F32, name="pr", tag="pr", bufs=2)
                    for g in range(4):
                        nc.tensor.matmul(
                            pr[:, g, :],
                            A[32 * g : 32 * g + KROWS, t_i, 128 * rt : 128 * (rt + 1)],
                            Bs[32 * g : 32 * g + KROWS, t_i, 512 * j : 512 * (j + 1)],
                            start=True,
                            stop=True,
                            tile_position=(32 * g, 0),
                        )
                    nc.scalar.copy(out=s[:, :, j, :], in_=pr)
                # row fold1: 1024 -> 512 within each batch, into the
                # 4-row-tile batch tile
                if rt % 4 == 0:
                    u = scratchp.tile(
                        [128, 4, 4, 512], BF16, name="u", tag="u", bufs=2
                    )
                nc.vector.tensor_tensor(
                    out=u[:, rt % 4], in0=s[:, :, 0, :], in1=s[:, :, 1, :], op=MAX
                )
                # col pairs / accumulators
                if rt % 2 == 0:
                    s_prev = s
                else:
                    p = scratchp.tile(
                        [128, 4, 2, 512], BF16, name="p", tag="p", bufs=3
                    )
                    nc.vector.tensor_tensor(out=p, in0=s_prev, in1=s, op=MAX)
                    pairs.append(p)
                if rt % 4 == 3:
                    half = rt // 4
                    acc = scratchp.tile(
                        [128, 4, 2, 512], BF16, name="acc", tag="acc", bufs=3
                    )
                    nc.vector.tensor_tensor(
                        out=acc, in0=pairs[2 * half], in1=pairs[2 * half + 1], op=MAX
                    )
                    nc.sync.dma_start(
                        out=col_d[ACC * t_i + half],
                        in_=acc.rearrange("p a b c -> p (a b c)"),
                    )
                    # row tail for row-tiles rt-3..rt (emitted after the
                    # col DMA so the final DMA overlaps this DVE work)
                    w = scratchp.tile(
                        [128, 4, 4, 256], BF16, name="w", tag="w", bufs=2
                    )
                    nc.vector.tensor_tensor(
                        out=w, in0=u[:, :, :, 0:256], in1=u[:, :, :, 256:512], op=MAX
                    )
                    x = scratchp.tile(
                        [128, 4, 4, 128], BF16, name="x", tag="x", bufs=2
                    )
                    nc.vector.tensor_tensor(
                        out=x, in0=w[:, :, :, 0:128], in1=w[:, :, :, 128:256], op=MAX
                    )
                    y = scratchp.tile(
                        [128, 4, 4, 64], BF16, name="y", tag="y", bufs=2
                    )
                    nc.vector.tensor_tensor(
                        out=y, in0=x[:, :, :, 0:64], in1=x[:, :, :, 64:128], op=MAX
                    )
                    nc.vector.tensor_reduce(
                        out=res_row[:, rt - 3 : rt + 1, 4 * t_i : 4 * t_i + 4],
                        in_=y,
                        axis=mybir.AxisListType.X,
                        op=MAX,
                    )

        nc.sync.dma_start(out=res_row_d, in_=res_row)


def _build_nc():
    if "nc" in _CACHE:
        return _CACHE["nc"]
    nc = bacc.Bacc(
        "TRN2", target_bir_lowering=False, debug=False, num_devices=NCORES
    )
    dram = {}
    for nm in ("ap_s", "bq_s"):
        dram[nm] = nc.dram_tensor(
            nm, (4, KROWS, NQUAD, 1024), BF16, kind="ExternalInput"
        ).ap()
    res_row_d = nc.dram_tensor(
        "res_row", (128, BPC * 8), BF16, kind="ExternalOutput"
    ).ap()
    col_d = nc.dram_tensor(
        "col_acc", (NQUAD * ACC, 128, 4096), BF16, kind="ExternalOutput"
    ).ap()
    with tile.TileContext(nc) as tc:
        _body(tc, dram, res_row_d, col_d)
    nc.compile()
    _CACHE["nc"] = nc
    return nc


def _split2(x):
    """Split fp32 into 2 bf16 terms (x ~= h + l, error ~2^-18 |x|)."""
    import ml_dtypes

    bf = ml_dtypes.bfloat16
    h = x.astype(bf)
    l = (x - h.astype(np.float32)).astype(bf)
    return h, l


def _host_stacks(x3, xn, lhs):
    """x3: (BPC, 1024, 3), xn: (BPC, 1024) -> (4, KROWS, NQUAD, 1024) bf16.

    Layout [g, k, t, n]: batch 4*t + g lives in PE row-group g (SBUF
    partitions 32g+k). With (h, l) the 2-level bf16 split, the K slots are
      cross (x3): lhsT [h h l], rhs [h l h]  (x3 comps each -> 9 rows)
      norms: lhsT [1 1 h(-xn/2) l], rhs [h(-yn/2) l 1 1]
    so sum_k lhsT[k]*rhs[k] = p.q - pn/2 - qn/2 = -dist_sq/2 (~2e-5 abs)."""
    import ml_dtypes

    bf = ml_dtypes.bfloat16
    out = np.empty((NQUAD, 4, KROWS, 1024), bf)  # [t, g, k, n]
    x3t = np.transpose(x3.reshape(NQUAD, 4, 1024, 3), (0, 1, 3, 2))  # (t,g,3,n)
    h3, l3 = _split2(x3t)
    hn, ln = _split2((xn * -0.5).reshape(NQUAD, 4, 1024))
    one = np.asarray(1.0, bf)
    if lhs:
        cross = (h3, h3, l3)
        norm = (one, one, hn, ln)
    else:
        cross = (h3, l3, h3)
        norm = (hn, ln, one, one)
    for s in range(3):
        out[:, :, 3 * s : 3 * s + 3] = cross[s]
    for s in range(4):
        out[:, :, 9 + s] = norm[s]
    return np.ascontiguousarray(np.transpose(out, (1, 2, 0, 3)))


def _decode_v(v):
    """bf16/f32 array of v = -dist_sq/2 -> float64 sum of distances."""
    d_sq = np.maximum(-2.0 * v.astype(np.float64), 0.0) + 1e-16
    return np.sqrt(d_sq).sum()


def _run(p, q, trace=False, tmpdir=None):
    import ml_dtypes

    p = np.asarray(p)
    q = np.asarray(q)
    assert p.shape == (B, N, 4) and q.shape == (B, M, 4)
    p3 = np.ascontiguousarray(p[:, :, 1:], dtype=np.float32)
    q3 = np.ascontiguousarray(q[:, :, 1:], dtype=np.float32)
    pn = np.einsum("bnc,bnc->bn", p3, p3)
    qn = np.einsum("bmc,bmc->bm", q3, q3)

    in_maps = []
    for c in range(NCORES):
        sl = slice(BPC * c, BPC * (c + 1))
        in_maps.append(
            {
                "ap_s": _host_stacks(p3[sl], pn[sl], lhs=True),
                "bq_s": _host_stacks(q3[sl], qn[sl], lhs=False),
            }
        )

    nc = _build_nc()
    kw = {}
    if trace:
        kw = {"trace": True, "tmpdir": tmpdir}
    rb = run_bass_kernel_spmd(nc, in_maps, core_ids=list(range(NCORES)), **kw)

    total = 0.0
    for c in range(NCORES):
        vrow = rb.results[c]["res_row"]  # (128, 128) bf16
        total += _decode_v(vrow)
        # col: for v <= 0 in bf16, float max across partitions ==
        # uint16 min (positives are ~1e-6 rounding artifacts; uint16-min
        # ranks them first which matches float max up to that noise)
        ca = np.asarray(rb.results[c]["col_acc"])  # (NQUAD*ACC, 128, 4096)
        u = ca.view(np.uint16).min(axis=1)  # (NQUAD*ACC, 4096)
        u = u.reshape(NQUAD, ACC, 4096).min(axis=1)  # (NQUAD, 4096)
        vcol = u.view(ml_dtypes.bfloat16)
        total += _decode_v(vcol)
    out = np.float32(total / 2.0)
    return out, rb


def kernel(p, q):
    out, _ = _run(p, q)
    return out
